# revision 20
# baseline (speedup 1.0000x reference)
"""Distributed Trainium2 Bass kernel for GQA attention (nn_Attention_27814208209106).

Sharding: 8 cores = 2 batches x 4 KV-head groups. No collectives:
  Phase 1: per-core q/k/v projections (7 q-heads + 1 kv head) + RoPE from
           host-transposed bf16 x^T. RoPE runs full-width (128 partitions):
           ScalarE swaps the psum halves, DVE does 2 muls + 1 add against
           host-duplicated [cos;cos] / [-sin;sin] tables. The first half of
           the v-projection is interleaved into the DMA-paced k-proj loop to
           keep PE busy (and HAM warm) while x^T streams in.
  Phase 2: causal attention in 512-wide T-blocks (k-stationary orientation,
           exp on ScalarE with 2-head software pipelining, denominators via
           ones-matmul), immediately followed by a PARTIAL o-proj: each core
           contracts its local 896 qkv dims against its 896-row slice of wo,
           producing a full-width [T, EMB] bf16 partial.
Host sums the 4 partials per batch (output unshard of a sum-sharded o-proj).

All matmuls in bf16 with f32 PSUM accumulation.
"""

import math
import numpy as np

import concourse.bass as bass
import concourse.mybir as mybir
import concourse.tile as tile
from concourse import bacc
from concourse.bass_utils import run_bass_kernel_spmd

P = 128
FB = 512  # psum free-dim block (f32 psum bank limit)
THETA = 1000000.0

F32 = mybir.dt.float32
BF16 = mybir.dt.bfloat16


class Cfg:
    def __init__(self, T=1024, EMB=3584, NH=28, KVH=4, HD=128):
        self.T, self.EMB, self.NH, self.KVH, self.HD = T, EMB, NH, KVH, HD
        self.GQ = NH // KVH          # q heads per kv head (7)
        self.HG = self.GQ * HD       # per-core q width (896)
        self.NHD = NH * HD           # full qkv width (3584)
        self.KT = EMB // P           # contraction tiles (28)
        self.TT = T // P             # token tiles (8)
        self.EB = EMB // FB          # o-proj free blocks (7)
        self.NB = (T + FB - 1) // FB  # 512-blocks of T
        self.scale = HD ** -0.5


def _t_blocks(cfg):
    """[(t0, w)] 512-aligned blocks covering [0, T)."""
    return [(b * FB, min(cfg.T, (b + 1) * FB) - b * FB) for b in range(cfg.NB)]


def build_kernel(cfg: Cfg):
    nc = bacc.Bacc(
        "TRN2",
        target_bir_lowering=False,
        debug=False,
        enable_asserts=False,
        num_devices=8,
    )

    # packed layouts (one contiguous [128, W] row-block per DMA)
    xb = nc.dram_tensor("xb", [P, cfg.KT * cfg.T], BF16, kind="ExternalInput").ap()
    wq_s = nc.dram_tensor("wq_s", [P, cfg.KT * cfg.HG], BF16, kind="ExternalInput").ap()
    wkv_s = nc.dram_tensor("wkv_s", [P, 2 * cfg.KT * cfg.HD + P], BF16, kind="ExternalInput").ap()
    wo_s = nc.dram_tensor("wo_s", [P, cfg.GQ * cfg.EMB], BF16, kind="ExternalInput").ap()
    cos2 = nc.dram_tensor("cos2", [P, cfg.T], F32, kind="ExternalInput").ap()
    sin2 = nc.dram_tensor("sin2", [P, cfg.T], F32, kind="ExternalInput").ap()
    o_s = nc.dram_tensor("o_s", [cfg.T, cfg.EMB], BF16, kind="ExternalOutput").ap()

    with tile.TileContext(nc) as tc:
        _body(tc, cfg, xb, wq_s, wkv_s, wo_s, cos2, sin2, o_s)

    nc.compile()
    return nc


def _body(tc, cfg, xb, wq_s, wkv_s, wo_s, cos2, sin2, o_s):
    nc = tc.nc
    H2 = cfg.HD // 2
    tblocks = _t_blocks(cfg)

    with (
        tc.tile_pool(name="const", bufs=1) as constp,
        tc.tile_pool(name="qT", bufs=cfg.GQ) as qTp,
        tc.tile_pool(name="kT", bufs=1) as kTp,
        tc.tile_pool(name="vv", bufs=cfg.TT) as vp,
    ):
        # --- constants (no GpSimd: avoids a ucode LIBRARY_RELOAD stall;
        # dmask ships from the host inside wkv_s) ---
        ones_bf = constp.tile([P, 1], BF16, name="ones_bf")
        nc.vector.memset(ones_bf, 1.0)
        dmask = constp.tile([P, P], BF16, name="dmask")
        # pre-warm the GpSimd broadcast ucode library (LIBRARY_RELOAD ~10us)
        # while the input DMAs stream, not mid-attention
        gwarm = constp.tile([P, 8], F32, name="gwarm")
        gsrc = constp.tile([1, 8], F32, name="gsrc")
        nc.vector.memset(gsrc, 1.0)
        wrm = constp.tile([P, FB], BF16, name="wrm")
        nc.vector.memset(wrm, 0.0)

        qT = [qTp.tile([P, cfg.T], BF16, name=f"qT{h}", tag="qT") for h in range(cfg.GQ)]
        kT = kTp.tile([P, cfg.T], BF16, name="kT")
        vts = [vp.tile([P, cfg.HD], BF16, name=f"v{i}", tag="v") for i in range(cfg.TT)]

        # ================= Phase 1: projections + RoPE =================
        with (
            tc.tile_pool(name="rope_cs", bufs=1) as csp,
            tc.tile_pool(name="xT", bufs=4) as xTp,
            tc.tile_pool(name="wqh", bufs=4) as wqhp,
            tc.tile_pool(name="wkvh", bufs=1) as wkvhp,
            tc.tile_pool(name="pproj", bufs=4, space="PSUM") as pprojp,
            tc.tile_pool(name="pv", bufs=4, space="PSUM") as pvp,
            tc.tile_pool(name="rtmp", bufs=4) as rtp,
        ):
            # PE warmup burst while the first DMAs stream in
            nc.gpsimd.partition_broadcast(gwarm, gsrc)
            psw = pprojp.tile([P, FB], F32, name="psw", tag="pproj")
            for _ in range(36):
                nc.tensor.matmul(out=psw, lhsT=wrm[:, 0:P], rhs=wrm, start=True, stop=True)

            # host-packed layouts: each DMA moves one contiguous [128, W]
            # row-block (12-50 KB rows) to amortize descriptor overhead.
            GG = 7  # ke-tiles per group DMA
            NG = cfg.KT // GG
            wkv_sb = wkvhp.tile([P, 2 * cfg.KT * cfg.HD + P], BF16, name="wkv_sb")
            nc.sync.dma_start(wkv_sb, wkv_s)
            wkh = [wkv_sb[:, ke * cfg.HD:(ke + 1) * cfg.HD] for ke in range(cfg.KT)]
            wvh = [wkv_sb[:, (cfg.KT + ke) * cfg.HD:(cfg.KT + ke + 1) * cfg.HD]
                   for ke in range(cfg.KT)]
            nc.vector.tensor_copy(
                dmask, wkv_sb[:, 2 * cfg.KT * cfg.HD:2 * cfg.KT * cfg.HD + P])
            xg = [xTp.tile([P, GG * cfg.T], BF16, name=f"xg{g}", tag="xT")
                  for g in range(NG)]
            for g in range(NG):
                nc.sync.dma_start(xg[g], xb[:, g * GG * cfg.T:(g + 1) * GG * cfg.T])
            xTt = [xg[ke // GG][:, (ke % GG) * cfg.T:(ke % GG + 1) * cfg.T]
                   for ke in range(cfg.KT)]
            cos_sb = csp.tile([P, cfg.T], F32, name="cos_sb")
            sin_sb = csp.tile([P, cfg.T], F32, name="sin_sb")
            nc.sync.dma_start(cos_sb, cos2)
            nc.sync.dma_start(sin_sb, sin2)
            qg = [wqhp.tile([P, GG * cfg.HG], BF16, name=f"qg{g}", tag="wqh")
                  for g in range(NG)]
            for g in range(NG):
                nc.sync.dma_start(qg[g], wq_s[:, g * GG * cfg.HG:(g + 1) * GG * cfg.HG])
            wqh = [qg[ke // GG][:, (ke % GG) * cfg.HG:(ke % GG + 1) * cfg.HG]
                   for ke in range(cfg.KT)]

            def rope_drain(psum, dst, t0, w, swap_eng=None):
                """dst[:, t0:t0+w] = rope(psum) ; psum [128, w] f32.

                Full-width: tmp = [p2;p1] (swap copy), then
                dst = psum*[c;c] + tmp*[-s;s] on DVE.
                """
                eng = swap_eng or nc.scalar
                cp = (lambda o, i: eng.copy(o, i)) if eng is nc.scalar \
                    else (lambda o, i: eng.tensor_copy(o, i))
                tmp = rtp.tile([P, FB], F32, name="tmp", tag="rt1")[:, :w]
                t1 = rtp.tile([P, FB], F32, name="t1", tag="rt2")[:, :w]
                cp(tmp[0:H2, :], psum[H2:P, :])
                cp(tmp[H2:P, :], psum[0:H2, :])
                nc.vector.tensor_mul(t1, psum, cos_sb[:, t0:t0 + w])
                nc.vector.tensor_mul(tmp, tmp, sin_sb[:, t0:t0 + w])
                nc.vector.tensor_add(dst[:, t0:t0 + w], t1, tmp)

            # k projection + first half of v interleaved (DMA-paced window)
            psk = [pprojp.tile([P, FB], F32, name=f"psk{i}", tag="pproj")[:, :w]
                   for i, (t0, w) in enumerate(tblocks)]
            psv = [pvp.tile([P, cfg.HD], F32, name=f"psv{ti}", tag="pv")
                   for ti in range(4)]
            for ke in range(cfg.KT):
                for i, (t0, w) in enumerate(tblocks):
                    nc.tensor.matmul(
                        out=psk[i], lhsT=wkh[ke], rhs=xTt[ke][:, t0:t0 + w],
                        start=(ke == 0), stop=(ke == cfg.KT - 1),
                    )
                for ti in range(4):
                    nc.tensor.matmul(
                        out=psv[ti], lhsT=xTt[ke][:, ti * P:(ti + 1) * P],
                        rhs=wvh[ke],
                        start=(ke == 0), stop=(ke == cfg.KT - 1),
                    )
            for i, (t0, w) in enumerate(tblocks):
                rope_drain(psk[i], kT, t0, w)
            for ti in range(4):
                nc.any.tensor_copy(vts[ti], psv[ti])

            # second half of v
            for ti in range(4, cfg.TT):
                ps = pvp.tile([P, cfg.HD], F32, name="psv", tag="pv")
                for ke in range(cfg.KT):
                    nc.tensor.matmul(
                        out=ps, lhsT=xTt[ke][:, ti * P:(ti + 1) * P], rhs=wvh[ke],
                        start=(ke == 0), stop=(ke == cfg.KT - 1),
                    )
                nc.any.tensor_copy(vts[ti], ps)

            # q projection: stationary wq tile reused across all t-blocks
            for h in range(cfg.GQ):
                pss = [pprojp.tile([P, FB], F32, name=f"psq{i}", tag="pproj")[:, :w]
                       for i, (t0, w) in enumerate(tblocks)]
                for ke in range(cfg.KT):
                    for i, (t0, w) in enumerate(tblocks):
                        nc.tensor.matmul(
                            out=pss[i],
                            lhsT=wqh[ke][:, h * P:(h + 1) * P],
                            rhs=xTt[ke][:, t0:t0 + w],
                            start=(ke == 0), stop=(ke == cfg.KT - 1),
                        )
                for i, (t0, w) in enumerate(tblocks):
                    rope_drain(pss[i], qT[h], t0, w,
                               swap_eng=nc.vector if h >= cfg.GQ - 2 else None)

        # ============ Phase 2: attention + partial o-proj ============
        with (
            tc.tile_pool(name="pos", bufs=2, space="PSUM") as posp,
            tc.tile_pool(name="pav", bufs=2, space="PSUM") as pavp,
            tc.tile_pool(name="pl", bufs=3, space="PSUM") as plp,
            tc.tile_pool(name="psums", bufs=1, space="PSUM") as psumsp,
            tc.tile_pool(name="woh", bufs=1) as wohp,
            tc.tile_pool(name="pt", bufs=33, space="SBUF") as ptp,
            tc.tile_pool(name="qkvb", bufs=2 * cfg.GQ) as qkvbp,
            tc.tile_pool(name="accs", bufs=4) as accp,
            tc.tile_pool(name="rec", bufs=4) as recp,
            tc.tile_pool(name="recb", bufs=4) as recbp,
            tc.tile_pool(name="osb", bufs=3) as osbp,
        ):
            # wo row-slices: one packed DMA behind the attention compute
            wo_sb = wohp.tile([P, cfg.GQ * cfg.EMB], BF16, name="wo_sb")
            nc.sync.dma_start(wo_sb, wo_s)
            woh = [wo_sb[:, kt * cfg.EMB:(kt + 1) * cfg.EMB] for kt in range(cfg.GQ)]

            for tb, (t0b, wb) in enumerate(tblocks):
                si_last = min(cfg.TT - 1, (t0b + wb - 1) // P)

                def logits_exp(h):
                    """Issue logits matmuls + exp for head h; return pt tiles.

                    si runs descending so the small diagonal tile's exp frees
                    its psum buf while the big tiles stream."""
                    pts = []
                    for si in range(si_last, -1, -1):
                        c0 = max(t0b, si * P)
                        cw = t0b + wb - c0
                        pl = plp.tile([P, FB], F32, name="pl", tag="pl")[:, :cw]
                        nc.tensor.matmul(
                            out=pl,
                            lhsT=kT[:, si * P:(si + 1) * P],
                            rhs=qT[h][:, c0:c0 + cw],
                            start=True, stop=True,
                        )
                        pt = ptp.tile([P, FB], BF16, name="pt", tag="pt")[:, :cw]
                        nc.scalar.activation(
                            pt, pl, mybir.ActivationFunctionType.Exp,
                            scale=cfg.scale,
                        )
                        if si * P >= t0b:
                            # diagonal tile: mask invalid (s > t) entries
                            nc.vector.tensor_mul(pt[:, 0:P], pt[:, 0:P], dmask)
                        pts.append((pt, c0, cw, si))
                    return pts

                def reduce_head(pts):
                    """Denominator + attn@V + normalize; return qkv^T tile.

                    pt tiles are pre-summed on DVE (bf16) so PE streams one
                    N=wb ones-matmul per head instead of one per key tile."""
                    acc = accp.tile([P, FB], BF16, name="acc", tag="acc")[:, :wb]
                    full = [e for e in pts if e[2] == wb]
                    rest = [e for e in pts if e[2] != wb]
                    if len(full) >= 2:
                        nc.vector.tensor_add(acc, full[0][0], full[1][0])
                        later = full[2:]
                    else:
                        nc.vector.tensor_copy(acc, full[0][0])
                        later = []
                    for pt, c0, cw, si in later:
                        nc.vector.tensor_add(acc, acc, pt)
                    for pt, c0, cw, si in rest:
                        o = c0 - t0b
                        nc.vector.tensor_add(acc[:, o:o + cw], acc[:, o:o + cw], pt)
                    sp = psumsp.tile([1, FB], F32, name="sums", tag="sums")[:, :wb]
                    nc.tensor.matmul(out=sp, lhsT=ones_bf, rhs=acc,
                                     start=True, stop=True)
                    rec = recp.tile([1, FB], F32, name="rec", tag="rec")[:, :wb]
                    nc.vector.reciprocal_approx_fast(out=rec, in_=sp)
                    recb = recbp.tile([P, FB], F32, name="recb", tag="recb")[:, :wb]
                    nc.gpsimd.partition_broadcast(recb, rec)

                    pav = pavp.tile([P, FB], F32, name="pav", tag="pav")[:, :wb]
                    for i, (pt, c0, cw, si) in enumerate(pts):
                        nc.tensor.matmul(
                            out=pav[:, c0 - t0b:c0 - t0b + cw],
                            lhsT=vts[si], rhs=pt,
                            start=(i == 0), stop=(i == len(pts) - 1),
                        )
                    qkvb = qkvbp.tile([P, FB], BF16, name="qkvb", tag="qkvb")[:, :wb]
                    nc.vector.tensor_mul(qkvb, pav, recb)
                    return qkvb

                # 2-head software pipeline: logits(h+1), logits(h+2) issue
                # before reduce(h) so ScalarE's exp latency hides under PE work.
                LA = 3
                qkvh = [None] * cfg.GQ
                pend = {}
                for h in range(cfg.GQ):
                    pend[h] = logits_exp(h)
                    if h >= LA:
                        qkvh[h - LA] = reduce_head(pend.pop(h - LA))
                for h in range(cfg.GQ - LA, cfg.GQ):
                    qkvh[h] = reduce_head(pend.pop(h))

                # partial o-proj over this core's 896 qkv dims, full EMB width
                for ti in range(wb // P):
                    last = (tb == len(tblocks) - 1) and (ti == wb // P - 1)
                    osb = osbp.tile([P, cfg.EMB], BF16, name="osb", tag="osb")
                    rows = o_s[t0b + ti * P:t0b + (ti + 1) * P, :]
                    for eb in range(cfg.EB):
                        pos = posp.tile([P, FB], F32, name="pos", tag="pos")
                        for kt in range(cfg.GQ):
                            nc.tensor.matmul(
                                out=pos,
                                lhsT=qkvh[kt][:, ti * P:(ti + 1) * P],
                                rhs=woh[kt][:, eb * FB:(eb + 1) * FB],
                                start=(kt == 0), stop=(kt == cfg.GQ - 1),
                            )
                        nc.vector.tensor_copy(osb[:, eb * FB:(eb + 1) * FB], pos)
                        if last:
                            # stream the final tile out per-eb so the tail DMA
                            # overlaps the remaining matmuls
                            nc.sync.dma_start(rows[:, eb * FB:(eb + 1) * FB],
                                              osb[:, eb * FB:(eb + 1) * FB])
                    if not last:
                        nc.sync.dma_start(rows, osb)


# ======================= host side =======================

_NC_CACHE = {}


def _get_nc(cfg_key=None):
    if cfg_key not in _NC_CACHE:
        _NC_CACHE[cfg_key] = build_kernel(Cfg())
    return _NC_CACHE[cfg_key]


def _rope_tables(segment_ids, cur_ind, T, HD):
    """Duplicated full-width tables: cos2=[cos;cos], sin2=[-sin;sin]."""
    valid = (np.asarray(segment_ids) != 0)
    pos = np.cumsum(valid, axis=-1) - 1 + int(cur_ind)  # [B, T]
    frac = 2.0 * np.arange(HD // 2, dtype=np.float64) / HD
    timescale = THETA ** frac
    ang = pos[..., None].astype(np.float64) / timescale  # [B, T, HD/2]
    cosT = np.transpose(np.cos(ang), (0, 2, 1)).astype(np.float32)  # [B, HD/2, T]
    sinT = np.transpose(np.sin(ang), (0, 2, 1)).astype(np.float32)
    cos2 = np.concatenate([cosT, cosT], axis=1)   # [B, HD, T]
    sin2 = np.concatenate([-sinT, sinT], axis=1)  # [B, HD, T]
    return cos2, sin2


def prepare_in_maps(inputs, cfg=None):
    import ml_dtypes
    bf16 = ml_dtypes.bfloat16
    cfg = cfg or Cfg()
    x = np.asarray(inputs["x"], dtype=np.float32)
    wq = np.asarray(inputs["wq"], dtype=np.float32).astype(bf16)
    wk = np.asarray(inputs["wk"], dtype=np.float32).astype(bf16)
    wv = np.asarray(inputs["wv"], dtype=np.float32).astype(bf16)
    wo = np.asarray(inputs["wo"], dtype=np.float32).astype(bf16)
    seg = np.asarray(inputs["segment_ids"])
    cur = int(np.asarray(inputs["cur_ind"]))

    B, T, EMB = x.shape
    assert (B, T, EMB) == (2, cfg.T, cfg.EMB)
    HG = cfg.HG
    cos2, sin2 = _rope_tables(seg, cur, T, cfg.HD)
    xT = np.ascontiguousarray(np.transpose(x, (0, 2, 1))).astype(bf16)  # [B, EMB, T]

    def pack_rows(a):
        """[KT*128, W] -> [128, KT*W]: tile ke -> cols [ke*W:(ke+1)*W]."""
        kt, w = a.shape[0] // 128, a.shape[1]
        return np.ascontiguousarray(
            np.transpose(a.reshape(kt, 128, w), (1, 0, 2)).reshape(128, kt * w))

    in_maps = []
    for c in range(8):
        b, j = c // 4, c % 4
        wkv = np.concatenate([wk[:, j * cfg.HD:(j + 1) * cfg.HD],
                              wv[:, j * cfg.HD:(j + 1) * cfg.HD]], axis=0)
        dmask = (np.arange(128)[:, None] <= np.arange(128)[None, :]).astype(bf16)
        wkv_p = np.concatenate([pack_rows(wkv), dmask], axis=1)
        in_maps.append({
            "xb": pack_rows(xT[b]),
            "wq_s": pack_rows(np.ascontiguousarray(wq[:, j * HG:(j + 1) * HG])),
            "wkv_s": np.ascontiguousarray(wkv_p),
            "wo_s": pack_rows(np.ascontiguousarray(wo[j * HG:(j + 1) * HG, :])),
            "cos2": np.ascontiguousarray(cos2[b]),
            "sin2": np.ascontiguousarray(sin2[b]),
        })
    return in_maps


def assemble_out(results, cfg=None):
    cfg = cfg or Cfg()
    out = np.zeros((2, cfg.T, cfg.EMB), np.float32)
    for c in range(8):
        b = c // 4
        out[b] += np.asarray(results[c]["o_s"], dtype=np.float32)
    return out


def kernel(**inputs):
    cfg = Cfg()
    in_maps = prepare_in_maps(inputs, cfg)
    nc = _get_nc()
    res = run_bass_kernel_spmd(nc, in_maps, core_ids=list(range(8)))
    return assemble_out(res.results, cfg)


# revision 23
# speedup vs baseline: 1.0088x; 1.0088x over previous
"""Distributed Trainium2 Bass kernel for GQA attention (nn_Attention_27814208209106).

Sharding: 8 cores = 2 batches x 4 KV-head groups. No collectives:
  Phase 1: per-core q/k/v projections (7 q-heads + 1 kv head) + RoPE from
           host-transposed bf16 x^T. RoPE runs full-width (128 partitions):
           ScalarE swaps the psum halves, DVE does 2 muls + 1 add against
           host-duplicated [cos;cos] / [-sin;sin] tables. The first half of
           the v-projection is interleaved into the DMA-paced k-proj loop to
           keep PE busy (and HAM warm) while x^T streams in.
  Phase 2: causal attention in 512-wide T-blocks (k-stationary orientation,
           exp on ScalarE with 2-head software pipelining, denominators via
           ones-matmul), immediately followed by a PARTIAL o-proj: each core
           contracts its local 896 qkv dims against its 896-row slice of wo,
           producing a full-width [T, EMB] bf16 partial.
Host sums the 4 partials per batch (output unshard of a sum-sharded o-proj).

All matmuls in bf16 with f32 PSUM accumulation.
"""

import math
import numpy as np

import concourse.bass as bass
import concourse.mybir as mybir
import concourse.tile as tile
from concourse import bacc
from concourse.bass_utils import run_bass_kernel_spmd

P = 128
FB = 512  # psum free-dim block (f32 psum bank limit)
THETA = 1000000.0

F32 = mybir.dt.float32
BF16 = mybir.dt.bfloat16


class Cfg:
    def __init__(self, T=1024, EMB=3584, NH=28, KVH=4, HD=128):
        self.T, self.EMB, self.NH, self.KVH, self.HD = T, EMB, NH, KVH, HD
        self.GQ = NH // KVH          # q heads per kv head (7)
        self.HG = self.GQ * HD       # per-core q width (896)
        self.NHD = NH * HD           # full qkv width (3584)
        self.KT = EMB // P           # contraction tiles (28)
        self.TT = T // P             # token tiles (8)
        self.EB = EMB // FB          # o-proj free blocks (7)
        self.NB = (T + FB - 1) // FB  # 512-blocks of T
        self.scale = HD ** -0.5


def _t_blocks(cfg):
    """[(t0, w)] 512-aligned blocks covering [0, T)."""
    return [(b * FB, min(cfg.T, (b + 1) * FB) - b * FB) for b in range(cfg.NB)]


def build_kernel(cfg: Cfg):
    nc = bacc.Bacc(
        "TRN2",
        target_bir_lowering=False,
        debug=False,
        enable_asserts=False,
        num_devices=8,
    )

    # packed layouts (one contiguous [128, W] row-block per DMA)
    xb = nc.dram_tensor("xb", [P, cfg.KT * cfg.T], BF16, kind="ExternalInput").ap()
    wq_s = nc.dram_tensor("wq_s", [P, cfg.KT * cfg.HG], BF16, kind="ExternalInput").ap()
    wkv_s = nc.dram_tensor("wkv_s", [P, 2 * cfg.KT * cfg.HD + 2 * P], BF16, kind="ExternalInput").ap()
    wo_s = nc.dram_tensor("wo_s", [P, cfg.GQ * cfg.EMB], BF16, kind="ExternalInput").ap()
    cos2 = nc.dram_tensor("cos2", [P, cfg.T], F32, kind="ExternalInput").ap()
    sin2 = nc.dram_tensor("sin2", [P, cfg.T], F32, kind="ExternalInput").ap()
    o_s = nc.dram_tensor("o_s", [cfg.T, cfg.EMB], BF16, kind="ExternalOutput").ap()

    with tile.TileContext(nc) as tc:
        _body(tc, cfg, xb, wq_s, wkv_s, wo_s, cos2, sin2, o_s)

    nc.compile()
    return nc


def _body(tc, cfg, xb, wq_s, wkv_s, wo_s, cos2, sin2, o_s):
    nc = tc.nc
    H2 = cfg.HD // 2
    tblocks = _t_blocks(cfg)

    with (
        tc.tile_pool(name="const", bufs=1) as constp,
        tc.tile_pool(name="qT", bufs=cfg.GQ) as qTp,
        tc.tile_pool(name="kT", bufs=1) as kTp,
        tc.tile_pool(name="vv", bufs=cfg.TT) as vp,
    ):
        # --- constants (no GpSimd: avoids a ucode LIBRARY_RELOAD stall;
        # dmask ships from the host inside wkv_s) ---
        ones_bf = constp.tile([P, 1], BF16, name="ones_bf")
        nc.vector.memset(ones_bf, 1.0)
        dmask = constp.tile([P, P], BF16, name="dmask")
        # pre-warm the GpSimd broadcast ucode library (LIBRARY_RELOAD ~10us)
        # while the input DMAs stream, not mid-attention
        gwarm = constp.tile([P, 8], F32, name="gwarm")
        gsrc = constp.tile([1, 8], F32, name="gsrc")
        nc.vector.memset(gsrc, 1.0)
        wrm = constp.tile([P, FB], BF16, name="wrm")
        nc.vector.memset(wrm, 0.0)

        qT = [qTp.tile([P, cfg.T], BF16, name=f"qT{h}", tag="qT") for h in range(cfg.GQ)]
        kT = kTp.tile([P, cfg.T], BF16, name="kT")
        vts = [vp.tile([P, cfg.HD], BF16, name=f"v{i}", tag="v") for i in range(cfg.TT)]

        # ================= Phase 1: projections + RoPE =================
        with (
            tc.tile_pool(name="rope_cs", bufs=1) as csp,
            tc.tile_pool(name="xT", bufs=4) as xTp,
            tc.tile_pool(name="wqh", bufs=4) as wqhp,
            tc.tile_pool(name="wkvh", bufs=1) as wkvhp,
            tc.tile_pool(name="pproj", bufs=4, space="PSUM") as pprojp,
            tc.tile_pool(name="pv", bufs=2, space="PSUM") as pvp,
            tc.tile_pool(name="ptr", bufs=2, space="PSUM") as ptrp,
            tc.tile_pool(name="rtmp", bufs=4) as rtp,
        ):
            # PE warmup burst while the first DMAs stream in
            nc.gpsimd.partition_broadcast(gwarm, gsrc)
            psw = pprojp.tile([P, FB], F32, name="psw", tag="pproj")
            for _ in range(36):
                nc.tensor.matmul(out=psw, lhsT=wrm[:, 0:P], rhs=wrm, start=True, stop=True)

            # host-packed layouts: each DMA moves one contiguous [128, W]
            # row-block (12-50 KB rows) to amortize descriptor overhead.
            GG = 7  # ke-tiles per group DMA
            NG = cfg.KT // GG
            wkv_sb = wkvhp.tile([P, 2 * cfg.KT * cfg.HD + 2 * P], BF16, name="wkv_sb")
            nc.sync.dma_start(wkv_sb, wkv_s)
            wkh = [wkv_sb[:, ke * cfg.HD:(ke + 1) * cfg.HD] for ke in range(cfg.KT)]
            wvh = [wkv_sb[:, (cfg.KT + ke) * cfg.HD:(cfg.KT + ke + 1) * cfg.HD]
                   for ke in range(cfg.KT)]
            nc.vector.tensor_copy(
                dmask, wkv_sb[:, 2 * cfg.KT * cfg.HD:2 * cfg.KT * cfg.HD + P])
            ident = wkv_sb[:, 2 * cfg.KT * cfg.HD + P:2 * cfg.KT * cfg.HD + 2 * P]
            xg = [xTp.tile([P, GG * cfg.T], BF16, name=f"xg{g}", tag="xT")
                  for g in range(NG)]
            for g in range(NG):
                nc.sync.dma_start(xg[g], xb[:, g * GG * cfg.T:(g + 1) * GG * cfg.T])
            xTt = [xg[ke // GG][:, (ke % GG) * cfg.T:(ke % GG + 1) * cfg.T]
                   for ke in range(cfg.KT)]
            cos_sb = csp.tile([P, cfg.T], F32, name="cos_sb")
            sin_sb = csp.tile([P, cfg.T], F32, name="sin_sb")
            nc.sync.dma_start(cos_sb, cos2)
            nc.sync.dma_start(sin_sb, sin2)
            qg = [wqhp.tile([P, GG * cfg.HG], BF16, name=f"qg{g}", tag="wqh")
                  for g in range(NG)]
            for g in range(NG):
                nc.sync.dma_start(qg[g], wq_s[:, g * GG * cfg.HG:(g + 1) * GG * cfg.HG])
            wqh = [qg[ke // GG][:, (ke % GG) * cfg.HG:(ke % GG + 1) * cfg.HG]
                   for ke in range(cfg.KT)]

            def rope_drain(psum, dst, t0, w, swap_eng=None):
                """dst[:, t0:t0+w] = rope(psum) ; psum [128, w] f32.

                Full-width: tmp = [p2;p1] (swap copy), then
                dst = psum*[c;c] + tmp*[-s;s] on DVE.
                """
                eng = swap_eng or nc.scalar
                cp = (lambda o, i: eng.copy(o, i)) if eng is nc.scalar \
                    else (lambda o, i: eng.tensor_copy(o, i))
                tmp = rtp.tile([P, FB], F32, name="tmp", tag="rt1")[:, :w]
                t1 = rtp.tile([P, FB], F32, name="t1", tag="rt2")[:, :w]
                cp(tmp[0:H2, :], psum[H2:P, :])
                cp(tmp[H2:P, :], psum[0:H2, :])
                nc.vector.tensor_mul(t1, psum, cos_sb[:, t0:t0 + w])
                nc.vector.tensor_mul(tmp, tmp, sin_sb[:, t0:t0 + w])
                nc.vector.tensor_add(dst[:, t0:t0 + w], t1, tmp)

            # k and v^T projections interleaved (DMA-paced window): both
            # hd-major, two N=512 matmuls each per ke
            psk = [pprojp.tile([P, FB], F32, name=f"psk{i}", tag="pproj")[:, :w]
                   for i, (t0, w) in enumerate(tblocks)]
            psv = [pvp.tile([P, FB], F32, name=f"psv{i}", tag="pv")[:, :w]
                   for i, (t0, w) in enumerate(tblocks)]
            for ke in range(cfg.KT):
                for i, (t0, w) in enumerate(tblocks):
                    nc.tensor.matmul(
                        out=psk[i], lhsT=wkh[ke], rhs=xTt[ke][:, t0:t0 + w],
                        start=(ke == 0), stop=(ke == cfg.KT - 1),
                    )
                for i, (t0, w) in enumerate(tblocks):
                    nc.tensor.matmul(
                        out=psv[i], lhsT=wvh[ke], rhs=xTt[ke][:, t0:t0 + w],
                        start=(ke == 0), stop=(ke == cfg.KT - 1),
                    )
            for i, (t0, w) in enumerate(tblocks):
                rope_drain(psk[i], kT, t0, w)
            # drain v^T to SBUF, then transpose per token tile on PE
            vT_sb = csp.tile([P, cfg.T], BF16, name="vT_sb")
            for i, (t0, w) in enumerate(tblocks):
                nc.any.tensor_copy(vT_sb[:, t0:t0 + w], psv[i])
            for ti in range(cfg.TT):
                tp = ptrp.tile([P, P], BF16, name="tp", tag="ptr")
                nc.tensor.transpose(tp, vT_sb[:, ti * P:(ti + 1) * P], ident)
                nc.any.tensor_copy(vts[ti], tp)

            # q projection: stationary wq tile reused across all t-blocks
            for h in range(cfg.GQ):
                pss = [pprojp.tile([P, FB], F32, name=f"psq{i}", tag="pproj")[:, :w]
                       for i, (t0, w) in enumerate(tblocks)]
                for ke in range(cfg.KT):
                    for i, (t0, w) in enumerate(tblocks):
                        nc.tensor.matmul(
                            out=pss[i],
                            lhsT=wqh[ke][:, h * P:(h + 1) * P],
                            rhs=xTt[ke][:, t0:t0 + w],
                            start=(ke == 0), stop=(ke == cfg.KT - 1),
                        )
                for i, (t0, w) in enumerate(tblocks):
                    rope_drain(pss[i], qT[h], t0, w,
                               swap_eng=nc.vector if h >= cfg.GQ - 2 else None)

        # ============ Phase 2: attention + partial o-proj ============
        with (
            tc.tile_pool(name="pos", bufs=2, space="PSUM") as posp,
            tc.tile_pool(name="pav", bufs=2, space="PSUM") as pavp,
            tc.tile_pool(name="pl", bufs=3, space="PSUM") as plp,
            tc.tile_pool(name="psums", bufs=1, space="PSUM") as psumsp,
            tc.tile_pool(name="woh", bufs=1) as wohp,
            tc.tile_pool(name="pt", bufs=24, space="SBUF") as ptp,
            tc.tile_pool(name="qkvb", bufs=2 * cfg.GQ) as qkvbp,
            tc.tile_pool(name="rec", bufs=4) as recp,
            tc.tile_pool(name="recb", bufs=4) as recbp,
            tc.tile_pool(name="osb", bufs=3) as osbp,
        ):
            # wo row-slices: one packed DMA behind the attention compute
            wo_sb = wohp.tile([P, cfg.GQ * cfg.EMB], BF16, name="wo_sb")
            nc.sync.dma_start(wo_sb, wo_s)
            woh = [wo_sb[:, kt * cfg.EMB:(kt + 1) * cfg.EMB] for kt in range(cfg.GQ)]

            for tb, (t0b, wb) in enumerate(tblocks):
                si_last = min(cfg.TT - 1, (t0b + wb - 1) // P)

                def logits_exp(h):
                    """Issue logits matmuls + exp for head h; return pt tiles.

                    si runs descending so the small diagonal tile's exp frees
                    its psum buf while the big tiles stream."""
                    pts = []
                    for si in range(si_last, -1, -1):
                        c0 = max(t0b, si * P)
                        cw = t0b + wb - c0
                        pl = plp.tile([P, FB], F32, name="pl", tag="pl")[:, :cw]
                        nc.tensor.matmul(
                            out=pl,
                            lhsT=kT[:, si * P:(si + 1) * P],
                            rhs=qT[h][:, c0:c0 + cw],
                            start=True, stop=True,
                        )
                        pt = ptp.tile([P, FB], BF16, name="pt", tag="pt")[:, :cw]
                        nc.scalar.activation(
                            pt, pl, mybir.ActivationFunctionType.Exp,
                            scale=cfg.scale,
                        )
                        if si * P >= t0b:
                            # diagonal tile: mask invalid (s > t) entries
                            nc.vector.tensor_mul(pt[:, 0:P], pt[:, 0:P], dmask)
                        pts.append((pt, c0, cw, si))
                    return pts

                def reduce_head(pts):
                    """Denominator + attn@V + normalize; return qkv^T tile."""
                    sp = psumsp.tile([1, FB], F32, name="sums", tag="sums")[:, :wb]
                    for i, (pt, c0, cw, si) in enumerate(pts):
                        nc.tensor.matmul(
                            out=sp[:, c0 - t0b:c0 - t0b + cw],
                            lhsT=ones_bf, rhs=pt,
                            start=(i == 0), stop=(i == len(pts) - 1),
                        )
                    rec = recp.tile([1, FB], F32, name="rec", tag="rec")[:, :wb]
                    nc.vector.reciprocal_approx_fast(out=rec, in_=sp)
                    recb = recbp.tile([P, FB], F32, name="recb", tag="recb")[:, :wb]
                    nc.gpsimd.partition_broadcast(recb, rec)

                    pav = pavp.tile([P, FB], F32, name="pav", tag="pav")[:, :wb]
                    for i, (pt, c0, cw, si) in enumerate(pts):
                        nc.tensor.matmul(
                            out=pav[:, c0 - t0b:c0 - t0b + cw],
                            lhsT=vts[si], rhs=pt,
                            start=(i == 0), stop=(i == len(pts) - 1),
                        )
                    qkvb = qkvbp.tile([P, FB], BF16, name="qkvb", tag="qkvb")[:, :wb]
                    nc.vector.tensor_mul(qkvb, pav, recb)
                    return qkvb

                # 2-head software pipeline: logits(h+1), logits(h+2) issue
                # before reduce(h) so ScalarE's exp latency hides under PE work.
                qkvh = [None] * cfg.GQ
                pend = {}
                for h in range(cfg.GQ):
                    pend[h] = logits_exp(h)
                    if h >= 2:
                        qkvh[h - 2] = reduce_head(pend.pop(h - 2))
                qkvh[cfg.GQ - 2] = reduce_head(pend.pop(cfg.GQ - 2))
                qkvh[cfg.GQ - 1] = reduce_head(pend.pop(cfg.GQ - 1))

                # partial o-proj over this core's 896 qkv dims, full EMB width
                for ti in range(wb // P):
                    last = (tb == len(tblocks) - 1) and (ti == wb // P - 1)
                    osb = osbp.tile([P, cfg.EMB], BF16, name="osb", tag="osb")
                    rows = o_s[t0b + ti * P:t0b + (ti + 1) * P, :]
                    for eb in range(cfg.EB):
                        pos = posp.tile([P, FB], F32, name="pos", tag="pos")
                        for kt in range(cfg.GQ):
                            nc.tensor.matmul(
                                out=pos,
                                lhsT=qkvh[kt][:, ti * P:(ti + 1) * P],
                                rhs=woh[kt][:, eb * FB:(eb + 1) * FB],
                                start=(kt == 0), stop=(kt == cfg.GQ - 1),
                            )
                        nc.vector.tensor_copy(osb[:, eb * FB:(eb + 1) * FB], pos)
                        if last:
                            # stream the final tile out per-eb so the tail DMA
                            # overlaps the remaining matmuls
                            nc.sync.dma_start(rows[:, eb * FB:(eb + 1) * FB],
                                              osb[:, eb * FB:(eb + 1) * FB])
                    if not last:
                        nc.sync.dma_start(rows, osb)


# ======================= host side =======================

_NC_CACHE = {}


def _get_nc(cfg_key=None):
    if cfg_key not in _NC_CACHE:
        _NC_CACHE[cfg_key] = build_kernel(Cfg())
    return _NC_CACHE[cfg_key]


def _rope_tables(segment_ids, cur_ind, T, HD):
    """Duplicated full-width tables: cos2=[cos;cos], sin2=[-sin;sin]."""
    valid = (np.asarray(segment_ids) != 0)
    pos = np.cumsum(valid, axis=-1) - 1 + int(cur_ind)  # [B, T]
    frac = 2.0 * np.arange(HD // 2, dtype=np.float64) / HD
    timescale = THETA ** frac
    ang = pos[..., None].astype(np.float64) / timescale  # [B, T, HD/2]
    cosT = np.transpose(np.cos(ang), (0, 2, 1)).astype(np.float32)  # [B, HD/2, T]
    sinT = np.transpose(np.sin(ang), (0, 2, 1)).astype(np.float32)
    cos2 = np.concatenate([cosT, cosT], axis=1)   # [B, HD, T]
    sin2 = np.concatenate([-sinT, sinT], axis=1)  # [B, HD, T]
    return cos2, sin2


def prepare_in_maps(inputs, cfg=None):
    import ml_dtypes
    bf16 = ml_dtypes.bfloat16
    cfg = cfg or Cfg()
    x = np.asarray(inputs["x"], dtype=np.float32)
    wq = np.asarray(inputs["wq"], dtype=np.float32).astype(bf16)
    wk = np.asarray(inputs["wk"], dtype=np.float32).astype(bf16)
    wv = np.asarray(inputs["wv"], dtype=np.float32).astype(bf16)
    wo = np.asarray(inputs["wo"], dtype=np.float32).astype(bf16)
    seg = np.asarray(inputs["segment_ids"])
    cur = int(np.asarray(inputs["cur_ind"]))

    B, T, EMB = x.shape
    assert (B, T, EMB) == (2, cfg.T, cfg.EMB)
    HG = cfg.HG
    cos2, sin2 = _rope_tables(seg, cur, T, cfg.HD)
    xT = np.ascontiguousarray(np.transpose(x, (0, 2, 1))).astype(bf16)  # [B, EMB, T]

    def pack_rows(a):
        """[KT*128, W] -> [128, KT*W]: tile ke -> cols [ke*W:(ke+1)*W]."""
        kt, w = a.shape[0] // 128, a.shape[1]
        return np.ascontiguousarray(
            np.transpose(a.reshape(kt, 128, w), (1, 0, 2)).reshape(128, kt * w))

    in_maps = []
    for c in range(8):
        b, j = c // 4, c % 4
        wkv = np.concatenate([wk[:, j * cfg.HD:(j + 1) * cfg.HD],
                              wv[:, j * cfg.HD:(j + 1) * cfg.HD]], axis=0)
        dmask = (np.arange(128)[:, None] <= np.arange(128)[None, :]).astype(bf16)
        ident = np.eye(128, dtype=bf16)
        wkv_p = np.concatenate([pack_rows(wkv), dmask, ident], axis=1)
        in_maps.append({
            "xb": pack_rows(xT[b]),
            "wq_s": pack_rows(np.ascontiguousarray(wq[:, j * HG:(j + 1) * HG])),
            "wkv_s": np.ascontiguousarray(wkv_p),
            "wo_s": pack_rows(np.ascontiguousarray(wo[j * HG:(j + 1) * HG, :])),
            "cos2": np.ascontiguousarray(cos2[b]),
            "sin2": np.ascontiguousarray(sin2[b]),
        })
    return in_maps


def assemble_out(results, cfg=None):
    cfg = cfg or Cfg()
    out = np.zeros((2, cfg.T, cfg.EMB), np.float32)
    for c in range(8):
        b = c // 4
        out[b] += np.asarray(results[c]["o_s"], dtype=np.float32)
    return out


def kernel(**inputs):
    cfg = Cfg()
    in_maps = prepare_in_maps(inputs, cfg)
    nc = _get_nc()
    res = run_bass_kernel_spmd(nc, in_maps, core_ids=list(range(8)))
    return assemble_out(res.results, cfg)
